# revision 1
# baseline (speedup 1.0000x reference)
"""Multi-head attention (12 heads, head_dim 64, RoPE, seq 1024) on 8 trn2 cores.

Sharding: pure data-parallel over the 16 (batch, row) units -> 2 per core.
No collectives. Each core runs the full per-unit attention:

  layouts (per (b,r) unit):
    xT   [din=768, s=1024]   (6 sbuf tiles [128,1024]) via PE transpose of x
    qT/kT[dout=768, s=1024]  (6 tiles; tile hp = heads 2hp (rows 0:64), 2hp+1)
    v    [s=1024, dout=768]  (8 tiles [128,768], bf16)
    scoresT[k, q] = kT_h.T @ qT_h per head (fp32r matmuls, K=64 row-paired
      via tile_position (0,0)/(64,0))
    softmax: exp on ACT (no max subtraction - scores are O(5)), probsT bf16;
      sums broadcast across partitions by an ones[128,64] matmul into the
      same psum bank as PV; normalize = full-width reciprocal + TT mult
    PV: col-paired M=64 bf16 matmuls tile_position (0,0)/(0,64) -> paired attnT
    out = attnT.T @ Wo per 128-row chunk -> natural [s, 768] -> DMA out

  This walrus build encodes at most ONE semaphore wait per instruction;
  _legalize_waits() hoists excess waits into preceding same-engine NoOps
  (engine program order preserves the synchronization semantics).

  biases: bq/bk applied in-kernel (per-partition bias in the psum copyback);
  bv/bo folded in on the host: out += bv @ Wo + bo (exact: sum(probs)=1).
  mask: all-ones fast path; any zero -> exact numpy fallback.
"""
import numpy as np

H = 768
NH = 12
HD = 64
S = 1024
P = 128
DT = H // P          # 6 din/dout tiles
ST = S // P          # 8 seq tiles
BR = 2               # (b,r) units per core
NCORES = 8
QC = 256             # q-chunk
NQC = S // QC        # 4
ROPE_BASE = 10000.0

_CACHE = {}


def _rope_tables():
    inv = 1.0 / (ROPE_BASE ** (np.arange(0, HD, 2, dtype=np.float64) / HD))  # [32]
    t = np.arange(S, dtype=np.float64)
    f = np.outer(inv, t)                      # [32, S]
    cos2 = np.zeros((P, S), dtype=np.float32)
    sins = np.zeros((P, S), dtype=np.float32)
    c = np.cos(f).astype(np.float32)
    s = np.sin(f).astype(np.float32)
    for p in range(P):
        cos2[p] = c[p % 32]
        sins[p] = -s[p % 32] if (p % 64) < 32 else s[p % 32]
    return cos2, sins


def _legalize_waits(nc):
    """This walrus encodes at most one sync wait per instruction: hoist
    excess waits onto preceding same-engine NoOps."""
    import concourse.mybir as mybir

    n = 0
    for f in nc.m.functions:
        for blk in f.blocks:
            new = []
            for inst in blk.instructions:
                si = inst.sync_info
                waits = list(si.on_wait) if si and si.on_wait else []
                if len(waits) > 1:
                    for i, w in enumerate(waits[:-1]):
                        nop = mybir.InstNoOp(
                            name=f"{inst.name}-wn{i}", ins=[], outs=[],
                            sync_info=mybir.SyncInfo(on_wait=[w], on_update=[]))
                        nop.engine = inst.engine
                        new.append(nop)
                        n += 1
                    inst.sync_info = mybir.SyncInfo(
                        on_wait=[waits[-1]],
                        on_update=list(si.on_update) if si.on_update else [])
                new.append(inst)
            blk.instructions = new
    return n


def _build():
    import concourse.bass as bass
    import concourse.mybir as mybir
    import concourse.tile as tile

    F32 = mybir.dt.float32
    F32R = mybir.dt.float32r
    BF16 = mybir.dt.bfloat16
    Exp = mybir.ActivationFunctionType.Exp
    MUL = mybir.AluOpType.mult
    ADD = mybir.AluOpType.add

    nc = bass.Bass()
    xs = nc.dram_tensor("xs", [BR, S, H], F32, kind="ExternalInput")
    wq = nc.dram_tensor("wq", [H, H], F32, kind="ExternalInput")
    wk = nc.dram_tensor("wk", [H, H], F32, kind="ExternalInput")
    wv = nc.dram_tensor("wv", [H, H], F32, kind="ExternalInput")
    wo = nc.dram_tensor("wo", [H, H], F32, kind="ExternalInput")
    bq = nc.dram_tensor("bq", [H], F32, kind="ExternalInput")
    bk = nc.dram_tensor("bk", [H], F32, kind="ExternalInput")
    cos2 = nc.dram_tensor("cos2", [P, S], F32, kind="ExternalInput")
    sins = nc.dram_tensor("sins", [P, S], F32, kind="ExternalInput")
    identm = nc.dram_tensor("identm", [P, P], F32, kind="ExternalInput")
    onesb = nc.dram_tensor("onesb", [P, 64], BF16, kind="ExternalInput")
    out = nc.dram_tensor("out", [BR, S, H], F32, kind="ExternalOutput")

    def f32r(ap):
        return ap.bitcast(F32R)

    with tile.TileContext(nc) as tc:
        with tc.tile_pool(name="const", bufs=1) as cpool, \
             tc.tile_pool(name="wpool", bufs=1) as wpool, \
             tc.tile_pool(name="xn", bufs=2) as xn_pool, \
             tc.tile_pool(name="xT", bufs=1) as xT_pool, \
             tc.tile_pool(name="qk", bufs=1) as qk_pool, \
             tc.tile_pool(name="rope", bufs=2) as rope_pool, \
             tc.tile_pool(name="vp", bufs=1) as v_pool, \
             tc.tile_pool(name="probs", bufs=3) as probs_pool, \
             tc.tile_pool(name="rec", bufs=1) as rec_pool, \
             tc.tile_pool(name="at", bufs=1) as at_pool, \
             tc.tile_pool(name="ot", bufs=1) as ot_pool, \
             tc.tile_pool(name="sc", bufs=2, space="PSUM") as sc_pool, \
             tc.tile_pool(name="pv", bufs=1, space="PSUM") as pv_pool, \
             tc.tile_pool(name="pj", bufs=3, space="PSUM") as ppj_pool:

            ident = cpool.tile([P, P], F32, tag="ident")
            nc.sync.dma_start(ident[:], identm[:])
            ones64 = cpool.tile([P, 64], BF16, tag="ones")
            nc.sync.dma_start(ones64[:], onesb[:])
            cos_sb = cpool.tile([P, S], F32, tag="cos")
            sin_sb = cpool.tile([P, S], F32, tag="sin")
            nc.sync.dma_start(cos_sb[:], cos2[:])
            nc.sync.dma_start(sin_sb[:], sins[:])
            bq_sb = cpool.tile([P, DT], F32, tag="bq")
            bk_sb = cpool.tile([P, DT], F32, tag="bk")
            nc.sync.dma_start(bq_sb[:], bq.rearrange("(t p) -> p t", p=P))
            nc.sync.dma_start(bk_sb[:], bk.rearrange("(t p) -> p t", p=P))

            w_sb = {}
            for name, w in (("q", wq), ("k", wk), ("v", wv), ("o", wo)):
                w_sb[name] = wpool.tile([P, DT, H], F32, tag=f"w{name}",
                                        name=f"w{name}")
                nc.sync.dma_start(
                    w_sb[name][:].bitcast(F32R),
                    w.rearrange("(t p) o -> p t o", p=P).bitcast(F32R))

            for br in range(BR):
                # ---------- phase P: x -> xT (PE transpose) ----------
                xT = []
                for dj in range(DT):
                    xT.append(xT_pool.tile([P, S], F32, tag=f"xT{dj}",
                                           name=f"xT{dj}_{br}"))
                for st in range(ST):
                    xn = xn_pool.tile([P, H], F32, tag="xn")
                    nc.sync.dma_start(xn[:], xs[br, st * P:(st + 1) * P, :])
                    for hf in range(2):
                        pt = ppj_pool.tile([P, 512], F32, tag="pj")
                        for dj3 in range(3):
                            dj = hf * 3 + dj3
                            nc.tensor.transpose(pt[:, dj3 * P:(dj3 + 1) * P],
                                                xn[:, dj * P:(dj + 1) * P], ident)
                        for dj3 in range(3):
                            dj = hf * 3 + dj3
                            nc.vector.tensor_copy(
                                xT[dj][:, st * P:(st + 1) * P].bitcast(F32R),
                                pt[:, dj3 * P:(dj3 + 1) * P])

                # ---------- v projection (bf16, natural layout) ----------
                v_sb = []
                for st in range(ST):
                    vt = v_pool.tile([P, H], BF16, tag=f"v{st}", name=f"v{st}_{br}")
                    v_sb.append(vt)
                    for nb in range(2):
                        c0 = nb * 384
                        pp = ppj_pool.tile([P, 512], F32, tag="pj")
                        for dj in range(DT):
                            nc.tensor.matmul(
                                pp[:, 0:384],
                                f32r(xT[dj][:, st * P:(st + 1) * P]),
                                f32r(w_sb["v"][:, dj, c0:c0 + 384]),
                                start=(dj == 0), stop=(dj == DT - 1))
                        nc.vector.tensor_copy(vt[:, c0:c0 + 384], pp[:, 0:384])

                # ---------- qT/kT projections + bias + RoPE ----------
                qkT = {}
                for name, b_sb in (("q", bq_sb), ("k", bk_sb)):
                    tiles = []
                    for tt in range(DT):
                        dst = qk_pool.tile([P, S], F32, tag=f"{name}T{tt}",
                                           name=f"{name}T{tt}_{br}")
                        tiles.append(dst)
                        for half in range(2):
                            pp = ppj_pool.tile([P, 512], F32, tag="pj")
                            for dj in range(DT):
                                nc.tensor.matmul(
                                    pp[:, 0:512],
                                    f32r(w_sb[name][:, dj, tt * P:(tt + 1) * P]),
                                    f32r(xT[dj][:, half * 512:(half + 1) * 512]),
                                    start=(dj == 0), stop=(dj == DT - 1))
                            nc.vector.tensor_scalar_add(
                                dst[:, half * 512:(half + 1) * 512].bitcast(F32R),
                                pp[:, 0:512], b_sb[:, tt:tt + 1])
                        # RoPE: dst = dst*cos + swap(dst)*sins
                        sw = rope_pool.tile([P, S], F32, tag="ropesw")
                        for hh2 in range(2):
                            b0 = hh2 * 64
                            nc.sync.dma_start(sw[b0:b0 + 32, :],
                                              dst[b0 + 32:b0 + 64, :])
                            nc.sync.dma_start(sw[b0 + 32:b0 + 64, :],
                                              dst[b0:b0 + 32, :])
                        nc.vector.tensor_tensor(sw[:], sw[:], sin_sb[:], MUL)
                        nc.vector.tensor_tensor(dst[:].bitcast(F32R), dst[:],
                                                cos_sb[:], MUL)
                        nc.vector.tensor_tensor(dst[:].bitcast(F32R), dst[:],
                                                sw[:], ADD)
                    qkT[name] = tiles

                # ---------- phase A: attention ----------
                for qc in range(NQC):
                    q0 = qc * QC
                    at = []
                    for hp in range(DT):
                        probs = []
                        for hh, base in ((0, 0), (1, 64)):
                            pr = probs_pool.tile([P, ST, QC], BF16, tag="probs",
                                                 name=f"pr{hh}")
                            probs.append(pr)
                            for gi in range(2):
                                sc_ps = sc_pool.tile([P, 1024], F32, tag="sc")
                                for i in range(4):
                                    kt = gi * 4 + i
                                    nc.tensor.matmul(
                                        sc_ps[:, i * QC:(i + 1) * QC],
                                        f32r(qkT["k"][hp][base:base + 64,
                                                          kt * P:(kt + 1) * P]),
                                        f32r(qkT["q"][hp][base:base + 64,
                                                          q0:q0 + QC]),
                                        start=True, stop=True,
                                        tile_position=(base, 0))
                                nc.scalar.activation(
                                    pr[:, gi * 4:(gi + 1) * 4, :], sc_ps[:],
                                    Exp, scale=0.125)
                        # PV (col-paired, bf16) + bcast sums in one psum bank
                        pvt = pv_pool.tile([P, 512], F32, tag="pv")
                        for kt in range(ST):
                            nc.tensor.matmul(
                                pvt[0:64, 0:QC],
                                v_sb[kt][:, (2 * hp) * HD:(2 * hp + 1) * HD],
                                probs[0][:, kt, :],
                                start=(kt == 0), stop=(kt == ST - 1),
                                tile_position=(0, 0))
                            nc.tensor.matmul(
                                pvt[64:128, 0:QC],
                                v_sb[kt][:, (2 * hp + 1) * HD:(2 * hp + 2) * HD],
                                probs[1][:, kt, :],
                                start=(kt == 0), stop=(kt == ST - 1),
                                tile_position=(0, 64))
                        for kt in range(ST):
                            nc.tensor.matmul(
                                pvt[0:64, QC:2 * QC], ones64[:], probs[0][:, kt, :],
                                start=(kt == 0), stop=(kt == ST - 1),
                                tile_position=(0, 0))
                            nc.tensor.matmul(
                                pvt[64:128, QC:2 * QC], ones64[:], probs[1][:, kt, :],
                                start=(kt == 0), stop=(kt == ST - 1),
                                tile_position=(0, 64))
                        # normalize + copy to paired attnT (sums already
                        # broadcast across partitions by the ones64 matmul)
                        rec = rec_pool.tile([P, QC], F32, tag="rec")
                        nc.vector.reciprocal(rec[:], pvt[:, QC:2 * QC])
                        att = at_pool.tile([P, QC], F32, tag=f"at{hp}",
                                           name=f"at{hp}")
                        at.append(att)
                        nc.vector.tensor_tensor(att[:].bitcast(F32R),
                                                pvt[:, 0:QC], rec[:], MUL)

                    # ---------- out projection for this q-chunk ----------
                    for sc2 in range(QC // P):
                        ot = ot_pool.tile([P, H], F32, tag="ot")
                        for nb in range(2):
                            c0 = nb * 384
                            po = ppj_pool.tile([P, 512], F32, tag="pj")
                            for dj in range(DT):
                                nc.tensor.matmul(
                                    po[:, 0:384],
                                    f32r(at[dj][:, sc2 * P:(sc2 + 1) * P]),
                                    f32r(w_sb["o"][:, dj, c0:c0 + 384]),
                                    start=(dj == 0), stop=(dj == DT - 1))
                            nc.vector.tensor_copy(ot[:, c0:c0 + 384], po[:, 0:384])
                        r0 = q0 + sc2 * P
                        nc.sync.dma_start(out[br, r0:r0 + P, :], ot[:])

    _legalize_waits(nc)
    return nc


def _get_nc():
    if "nc" not in _CACHE:
        _CACHE["nc"] = _build()
    return _CACHE["nc"]


def _numpy_reference(x, Wq, bq, Wk, bk, Wv, bv, Wo, bo, mask):
    b, r, s, d = x.shape
    inv = 1.0 / (ROPE_BASE ** (np.arange(0, HD, 2, dtype=np.float32) / HD))
    t = np.arange(s, dtype=np.float32)
    f = np.outer(t, inv)
    emb = np.concatenate([f, f], axis=-1)
    cos, sin = np.cos(emb), np.sin(emb)

    def proj(W, bvec):
        y = x @ W + bvec
        return y.reshape(b, r, s, NH, HD).transpose(0, 1, 3, 2, 4)

    def rot(z):
        z1, z2 = z[..., :HD // 2], z[..., HD // 2:]
        return np.concatenate([-z2, z1], axis=-1)

    q = proj(Wq, bq)
    k = proj(Wk, bk)
    v = proj(Wv, bv)
    q = q * cos + rot(q) * sin
    k = k * cos + rot(k) * sin
    scores = np.einsum("brhqd,brhkd->brhqk", q, k) / np.sqrt(np.float32(HD))
    scores = np.where(mask == 0, -np.inf, scores)
    m = scores.max(axis=-1, keepdims=True)
    e = np.exp(scores - m)
    probs = e / e.sum(axis=-1, keepdims=True)
    o = np.einsum("brhqk,brhkd->brhqd", probs, v)
    o = o.transpose(0, 1, 3, 2, 4).reshape(b, r, s, d)
    return (o @ Wo + bo).astype(np.float32)


def _run(inputs, trace=False):
    import ml_dtypes
    from concourse.bass_utils import run_bass_kernel_spmd

    x = np.asarray(inputs["x"], dtype=np.float32)
    Wq = np.ascontiguousarray(np.asarray(inputs["Wq"], dtype=np.float32))
    Wk = np.ascontiguousarray(np.asarray(inputs["Wk"], dtype=np.float32))
    Wv = np.ascontiguousarray(np.asarray(inputs["Wv"], dtype=np.float32))
    Wo = np.ascontiguousarray(np.asarray(inputs["Wo"], dtype=np.float32))
    bq = np.asarray(inputs["bq"], dtype=np.float32)
    bk = np.asarray(inputs["bk"], dtype=np.float32)
    bv = np.asarray(inputs["bv"], dtype=np.float32)
    bo = np.asarray(inputs["bo"], dtype=np.float32)

    xf = np.ascontiguousarray(x.reshape(NCORES * BR, S, H))
    cos2, sins = _rope_tables()
    identm = np.eye(P, dtype=np.float32)
    onesb = np.ones((P, 64), dtype=ml_dtypes.bfloat16)
    nc = _get_nc()
    in_maps = []
    for c in range(NCORES):
        in_maps.append(dict(
            xs=np.ascontiguousarray(xf[c * BR:(c + 1) * BR]),
            wq=Wq, wk=Wk, wv=Wv, wo=Wo, bq=bq, bk=bk,
            cos2=cos2, sins=sins, identm=identm, onesb=onesb))
    res = run_bass_kernel_spmd(nc, in_maps, core_ids=list(range(NCORES)),
                               trace=trace)
    outs = np.concatenate([r["out"] for r in res.results], axis=0)
    out = outs.reshape(2, NCORES * BR // 2, S, H)
    out = out + (bv @ Wo + bo)
    return out.astype(np.float32), res


def kernel(**inputs):
    mask = np.asarray(inputs["mask"])
    if not np.all(mask != 0):
        return _numpy_reference(
            x=np.asarray(inputs["x"], np.float32),
            Wq=np.asarray(inputs["Wq"], np.float32),
            bq=np.asarray(inputs["bq"], np.float32),
            Wk=np.asarray(inputs["Wk"], np.float32),
            bk=np.asarray(inputs["bk"], np.float32),
            Wv=np.asarray(inputs["Wv"], np.float32),
            bv=np.asarray(inputs["bv"], np.float32),
            Wo=np.asarray(inputs["Wo"], np.float32),
            bo=np.asarray(inputs["bo"], np.float32),
            mask=mask)
    out, _ = _run(inputs, trace=False)
    return out



# revision 4
# speedup vs baseline: 1.1879x; 1.1879x over previous
"""Multi-head attention (12 heads, head_dim 64, RoPE, seq 1024) on 8 trn2 cores.

Sharding: pure data-parallel over the 16 (batch, row) units -> 2 per core.
No collectives. Each core runs the full per-unit attention.

v2: full-bf16 matmul path (FWL weight loads), QC=512, scores matmuls
alternate row groups (0,0)/(64,0) so adjacent pairs overlap in the PE
array, PV/ones col-paired at (0,0)/(0,64), and the attention loop is
software-pipelined: PV/normalize for iteration i-1 is emitted after the
scores+exp of iteration i so the PE never idles waiting on the ACT
engine's exp stream.

  layouts (per (b,r) unit, all bf16 unless noted):
    xT   [128, 6, 1024]  din-major transpose of x (PE transpose)
    qT/kT[128, 6, 1024]  tile hp holds heads 2hp (rows 0:64), 2hp+1
    v    8 x [128, 768]  natural [s, dout]
    scoresT[k, q] per head; exp on ACT (scale=1/8, no max subtraction);
    sums broadcast across partitions by ones[128,64] matmuls into the
    same psum tile as PV; normalize = reciprocal + TT mult
    out = attT.T @ Wo per 128-row chunk -> [s, 768] fp32 -> DMA out

  biases: bq/bk applied in-kernel; bv/bo folded in on the host:
  out += bv @ Wo + bo (exact: sum(probs)=1).
  mask: all-ones fast path; any zero -> exact numpy fallback.
"""
import numpy as np

H = 768
NH = 12
HD = 64
S = 1024
P = 128
DT = H // P          # 6 din/dout tiles
ST = S // P          # 8 seq tiles
BR = 2               # (b,r) units per core
NCORES = 8
QC = 512             # q-chunk
NQC = S // QC        # 2
ROPE_BASE = 10000.0

_CACHE = {}


def _rope_tables():
    inv = 1.0 / (ROPE_BASE ** (np.arange(0, HD, 2, dtype=np.float64) / HD))  # [32]
    t = np.arange(S, dtype=np.float64)
    f = np.outer(inv, t)                      # [32, S]
    cos2 = np.zeros((P, S), dtype=np.float32)
    sins = np.zeros((P, S), dtype=np.float32)
    c = np.cos(f).astype(np.float32)
    s = np.sin(f).astype(np.float32)
    for p in range(P):
        cos2[p] = c[p % 32]
        sins[p] = -s[p % 32] if (p % 64) < 32 else s[p % 32]
    return cos2, sins


def _legalize_waits(nc):
    """This walrus encodes at most one sync wait per instruction: hoist
    excess waits onto preceding same-engine NoOps."""
    import concourse.mybir as mybir

    n = 0
    for f in nc.m.functions:
        for blk in f.blocks:
            new = []
            for inst in blk.instructions:
                si = inst.sync_info
                waits = list(si.on_wait) if si and si.on_wait else []
                if len(waits) > 1:
                    for i, w in enumerate(waits[:-1]):
                        nop = mybir.InstNoOp(
                            name=f"{inst.name}-wn{i}", ins=[], outs=[],
                            sync_info=mybir.SyncInfo(on_wait=[w], on_update=[]))
                        nop.engine = inst.engine
                        new.append(nop)
                        n += 1
                    inst.sync_info = mybir.SyncInfo(
                        on_wait=[waits[-1]],
                        on_update=list(si.on_update) if si.on_update else [])
                new.append(inst)
            blk.instructions = new
    return n


def _build():
    import concourse.bass as bass
    import concourse.mybir as mybir
    import concourse.tile as tile

    F32 = mybir.dt.float32
    BF16 = mybir.dt.bfloat16
    Exp = mybir.ActivationFunctionType.Exp
    MUL = mybir.AluOpType.mult
    ADD = mybir.AluOpType.add

    nc = bass.Bass()
    xs = nc.dram_tensor("xs", [BR, S, H], BF16, kind="ExternalInput")
    wq = nc.dram_tensor("wq", [H, H], BF16, kind="ExternalInput")
    wk = nc.dram_tensor("wk", [H, H], BF16, kind="ExternalInput")
    wv = nc.dram_tensor("wv", [H, H], BF16, kind="ExternalInput")
    wo = nc.dram_tensor("wo", [H, H], BF16, kind="ExternalInput")
    bq = nc.dram_tensor("bq", [H], F32, kind="ExternalInput")
    bk = nc.dram_tensor("bk", [H], F32, kind="ExternalInput")
    cos2 = nc.dram_tensor("cos2", [P, S], BF16, kind="ExternalInput")
    sins = nc.dram_tensor("sins", [P, S], BF16, kind="ExternalInput")
    identm = nc.dram_tensor("identm", [P, P], BF16, kind="ExternalInput")
    onesb = nc.dram_tensor("onesb", [P, 64], BF16, kind="ExternalInput")
    out = nc.dram_tensor("out", [BR, S, H], F32, kind="ExternalOutput")

    with tile.TileContext(nc) as tc:
        with tc.tile_pool(name="const", bufs=1) as cpool, \
             tc.tile_pool(name="wpool", bufs=1) as wpool, \
             tc.tile_pool(name="xn", bufs=2) as xn_pool, \
             tc.tile_pool(name="xT", bufs=1) as xT_pool, \
             tc.tile_pool(name="qk", bufs=1) as qk_pool, \
             tc.tile_pool(name="rope", bufs=2) as rope_pool, \
             tc.tile_pool(name="vp", bufs=1) as v_pool, \
             tc.tile_pool(name="probs", bufs=2) as probs_pool, \
             tc.tile_pool(name="rec", bufs=2) as rec_pool, \
             tc.tile_pool(name="at", bufs=2) as at_pool, \
             tc.tile_pool(name="ot", bufs=2) as ot_pool, \
             tc.tile_pool(name="ps", bufs=3, space="PSUM") as ps_pool, \
             tc.tile_pool(name="pv", bufs=1, space="PSUM") as pv_pool:

            ident = cpool.tile([P, P], BF16, tag="ident")
            nc.sync.dma_start(ident[:], identm[:])
            ones64 = cpool.tile([P, 64], BF16, tag="ones")
            nc.sync.dma_start(ones64[:], onesb[:])
            cos_sb = cpool.tile([P, S], BF16, tag="cos")
            sin_sb = cpool.tile([P, S], BF16, tag="sin")
            nc.sync.dma_start(cos_sb[:], cos2[:])
            nc.sync.dma_start(sin_sb[:], sins[:])
            bq_sb = cpool.tile([P, DT], F32, tag="bq")
            bk_sb = cpool.tile([P, DT], F32, tag="bk")
            nc.sync.dma_start(bq_sb[:], bq.rearrange("(t p) -> p t", p=P))
            nc.sync.dma_start(bk_sb[:], bk.rearrange("(t p) -> p t", p=P))

            w_sb = {}
            for name, w in (("q", wq), ("k", wk), ("v", wv), ("o", wo)):
                w_sb[name] = wpool.tile([P, DT, H], BF16, tag=f"w{name}",
                                        name=f"w{name}")
                nc.sync.dma_start(
                    w_sb[name][:], w.rearrange("(t p) o -> p t o", p=P))

            for br in range(BR):
                # ---------- phase P: x -> xT (PE transpose) ----------
                xT = xT_pool.tile([P, DT, S], BF16, tag="xT", name=f"xT_{br}")
                for st in range(ST):
                    xn = xn_pool.tile([P, H], BF16, tag="xn")
                    nc.sync.dma_start(xn[:], xs[br, st * P:(st + 1) * P, :])
                    pt = ps_pool.tile([P, 1024], BF16, tag="ps")
                    for dj in range(DT):
                        nc.tensor.transpose(pt[:, dj * P:(dj + 1) * P],
                                            xn[:, dj * P:(dj + 1) * P], ident)
                    nc.vector.tensor_copy(
                        xT[:, :, st * P:(st + 1) * P],
                        pt[:, 0:DT * P].rearrange("p (t c) -> p t c", c=P))

                # ---------- v projection (natural layout) ----------
                v_sb = []
                for st in range(ST):
                    vt = v_pool.tile([P, H], BF16, tag=f"v{st}", name=f"v{st}_{br}")
                    v_sb.append(vt)
                    pp = ps_pool.tile([P, 2, 512], F32, tag="ps")
                    for nb in range(2):
                        c0 = nb * 384
                        for dj in range(DT):
                            nc.tensor.matmul(
                                pp[:, nb, 0:384],
                                xT[:, dj, st * P:(st + 1) * P],
                                w_sb["v"][:, dj, c0:c0 + 384],
                                start=(dj == 0), stop=(dj == DT - 1))
                    nc.vector.tensor_copy(
                        vt[:].rearrange("p (n c) -> p n c", n=2),
                        pp[:, :, 0:384])

                # ---------- qT/kT projections + bias + RoPE ----------
                qkT = {}
                for name, b_sb in (("q", bq_sb), ("k", bk_sb)):
                    dst = qk_pool.tile([P, DT, S], BF16, tag=f"{name}T",
                                       name=f"{name}T_{br}")
                    qkT[name] = dst
                    for tt in range(DT):
                        pp = ps_pool.tile([P, 2, 512], F32, tag="ps")
                        for half in range(2):
                            for dj in range(DT):
                                nc.tensor.matmul(
                                    pp[:, half, :],
                                    w_sb[name][:, dj, tt * P:(tt + 1) * P],
                                    xT[:, dj, half * 512:(half + 1) * 512],
                                    start=(dj == 0), stop=(dj == DT - 1))
                        nc.vector.tensor_scalar_add(
                            dst[:, tt, :].rearrange("p (h c) -> p h c", h=2),
                            pp[:], b_sb[:, tt:tt + 1])
                        # RoPE: dst = dst*cos + swap(dst)*sins
                        sw = rope_pool.tile([P, S], BF16, tag="ropesw")
                        for hh2 in range(2):
                            b0 = hh2 * 64
                            nc.sync.dma_start(sw[b0:b0 + 32, :],
                                              dst[b0 + 32:b0 + 64, tt, :])
                            nc.sync.dma_start(sw[b0 + 32:b0 + 64, :],
                                              dst[b0:b0 + 32, tt, :])
                        nc.vector.tensor_tensor(sw[:], sw[:], sin_sb[:], MUL)
                        nc.vector.tensor_tensor(dst[:, tt, :], dst[:, tt, :],
                                                cos_sb[:], MUL)
                        nc.vector.tensor_tensor(dst[:, tt, :], dst[:, tt, :],
                                                sw[:], ADD)

                # ---------- phase A: attention (software-pipelined) ----
                def emit_scores(qc, hp):
                    q0 = qc * QC
                    prs = []
                    for hh in range(2):
                        prs.append(probs_pool.tile([P, ST, QC], BF16,
                                                   tag=f"pr{hh}",
                                                   name=f"pr{hh}"))
                    for g in range(4):
                        scs = [ps_pool.tile([P, 2, QC], F32, tag="ps",
                                            name=f"sc{i}")
                               for i in range(2)]
                        for i in range(2):
                            kt = 2 * g + i
                            for hh, base in ((0, 0), (1, 64)):
                                nc.tensor.matmul(
                                    scs[hh][:, i, :],
                                    qkT["k"][base:base + 64, hp,
                                             kt * P:(kt + 1) * P],
                                    qkT["q"][base:base + 64, hp, q0:q0 + QC],
                                    start=True, stop=True,
                                    tile_position=(base, 0))
                        for hh in range(2):
                            nc.scalar.activation(
                                prs[hh][:, 2 * g:2 * g + 2, :], scs[hh][:],
                                Exp, scale=0.125)
                    return prs

                def emit_pv(qc, hp, prs, at):
                    # PV + bcast sums in one 2-bank psum tile
                    pvt = pv_pool.tile([P, 2, QC], F32, tag="pv")
                    for kt in range(ST):
                        for hh in range(2):
                            nc.tensor.matmul(
                                pvt[hh * 64:(hh + 1) * 64, 0, :],
                                v_sb[kt][:, (2 * hp + hh) * HD:
                                         (2 * hp + hh + 1) * HD],
                                prs[hh][:, kt, :],
                                start=(kt == 0), stop=(kt == ST - 1),
                                tile_position=(0, hh * 64))
                    for kt in range(ST):
                        for hh in range(2):
                            nc.tensor.matmul(
                                pvt[hh * 64:(hh + 1) * 64, 1, :],
                                ones64[:], prs[hh][:, kt, :],
                                start=(kt == 0), stop=(kt == ST - 1),
                                tile_position=(0, hh * 64))
                    rec = rec_pool.tile([P, QC], F32, tag="rec")
                    nc.vector.reciprocal(rec[:], pvt[:, 1, :])
                    att = at_pool.tile([P, QC], BF16, tag=f"at{hp}",
                                       name=f"at{hp}")
                    at[hp] = att
                    nc.vector.tensor_tensor(att[:], pvt[:, 0, :], rec[:], MUL)

                def emit_outproj(qc, at):
                    q0 = qc * QC
                    for sc2 in range(QC // P):
                        po = ps_pool.tile([P, 2, 512], F32, tag="ps")
                        for nb in range(2):
                            c0 = nb * 384
                            for dj in range(DT):
                                nc.tensor.matmul(
                                    po[:, nb, 0:384],
                                    at[dj][:, sc2 * P:(sc2 + 1) * P],
                                    w_sb["o"][:, dj, c0:c0 + 384],
                                    start=(dj == 0), stop=(dj == DT - 1))
                        ot = ot_pool.tile([P, H], F32, tag="ot")
                        nc.vector.tensor_copy(
                            ot[:].rearrange("p (n c) -> p n c", n=2),
                            po[:, :, 0:384])
                        r0 = q0 + sc2 * P
                        nc.sync.dma_start(out[br, r0:r0 + P, :], ot[:])

                iters = [(qc, hp) for qc in range(NQC) for hp in range(DT)]
                prev = None
                at_by_qc = {qc: {} for qc in range(NQC)}
                for (qc, hp) in iters:
                    prs = emit_scores(qc, hp)
                    if prev is not None:
                        pqc, php, pprs = prev
                        emit_pv(pqc, php, pprs, at_by_qc[pqc])
                        if php == DT - 1:
                            emit_outproj(pqc, at_by_qc[pqc])
                    prev = (qc, hp, prs)
                pqc, php, pprs = prev
                emit_pv(pqc, php, pprs, at_by_qc[pqc])
                emit_outproj(pqc, at_by_qc[pqc])

    _legalize_waits(nc)
    return nc


def _get_nc():
    if "nc" not in _CACHE:
        _CACHE["nc"] = _build()
    return _CACHE["nc"]


def _numpy_reference(x, Wq, bq, Wk, bk, Wv, bv, Wo, bo, mask):
    b, r, s, d = x.shape
    inv = 1.0 / (ROPE_BASE ** (np.arange(0, HD, 2, dtype=np.float32) / HD))
    t = np.arange(s, dtype=np.float32)
    f = np.outer(t, inv)
    emb = np.concatenate([f, f], axis=-1)
    cos, sin = np.cos(emb), np.sin(emb)

    def proj(W, bvec):
        y = x @ W + bvec
        return y.reshape(b, r, s, NH, HD).transpose(0, 1, 3, 2, 4)

    def rot(z):
        z1, z2 = z[..., :HD // 2], z[..., HD // 2:]
        return np.concatenate([-z2, z1], axis=-1)

    q = proj(Wq, bq)
    k = proj(Wk, bk)
    v = proj(Wv, bv)
    q = q * cos + rot(q) * sin
    k = k * cos + rot(k) * sin
    scores = np.einsum("brhqd,brhkd->brhqk", q, k) / np.sqrt(np.float32(HD))
    scores = np.where(mask == 0, -np.inf, scores)
    m = scores.max(axis=-1, keepdims=True)
    e = np.exp(scores - m)
    probs = e / e.sum(axis=-1, keepdims=True)
    o = np.einsum("brhqk,brhkd->brhqd", probs, v)
    o = o.transpose(0, 1, 3, 2, 4).reshape(b, r, s, d)
    return (o @ Wo + bo).astype(np.float32)


def _run(inputs, trace=False):
    import ml_dtypes
    from concourse.bass_utils import run_bass_kernel_spmd

    BF = ml_dtypes.bfloat16
    x = np.asarray(inputs["x"], dtype=np.float32)
    Wq = np.ascontiguousarray(np.asarray(inputs["Wq"], dtype=np.float32))
    Wk = np.ascontiguousarray(np.asarray(inputs["Wk"], dtype=np.float32))
    Wv = np.ascontiguousarray(np.asarray(inputs["Wv"], dtype=np.float32))
    Wo = np.ascontiguousarray(np.asarray(inputs["Wo"], dtype=np.float32))
    bq = np.asarray(inputs["bq"], dtype=np.float32)
    bk = np.asarray(inputs["bk"], dtype=np.float32)
    bv = np.asarray(inputs["bv"], dtype=np.float32)
    bo = np.asarray(inputs["bo"], dtype=np.float32)

    xf = np.ascontiguousarray(x.reshape(NCORES * BR, S, H).astype(BF))
    cos2, sins = _rope_tables()
    cos2 = cos2.astype(BF)
    sins = sins.astype(BF)
    identm = np.eye(P, dtype=np.float32).astype(BF)
    onesb = np.ones((P, 64), dtype=BF)
    wqb, wkb, wvb, wob = (np.ascontiguousarray(w.astype(BF))
                          for w in (Wq, Wk, Wv, Wo))
    nc = _get_nc()
    in_maps = []
    for c in range(NCORES):
        in_maps.append(dict(
            xs=np.ascontiguousarray(xf[c * BR:(c + 1) * BR]),
            wq=wqb, wk=wkb, wv=wvb, wo=wob, bq=bq, bk=bk,
            cos2=cos2, sins=sins, identm=identm, onesb=onesb))
    res = run_bass_kernel_spmd(nc, in_maps, core_ids=list(range(NCORES)),
                               trace=trace)
    outs = np.concatenate([r["out"] for r in res.results], axis=0)
    out = outs.reshape(2, NCORES * BR // 2, S, H)
    out = out + (bv @ Wo + bo)
    return out.astype(np.float32), res


def kernel(**inputs):
    mask = np.asarray(inputs["mask"])
    if not np.all(mask != 0):
        return _numpy_reference(
            x=np.asarray(inputs["x"], np.float32),
            Wq=np.asarray(inputs["Wq"], np.float32),
            bq=np.asarray(inputs["bq"], np.float32),
            Wk=np.asarray(inputs["Wk"], np.float32),
            bk=np.asarray(inputs["bk"], np.float32),
            Wv=np.asarray(inputs["Wv"], np.float32),
            bv=np.asarray(inputs["bv"], np.float32),
            Wo=np.asarray(inputs["Wo"], np.float32),
            bo=np.asarray(inputs["bo"], np.float32),
            mask=mask)
    out, _ = _run(inputs, trace=False)
    return out


# revision 6
# speedup vs baseline: 1.5090x; 1.2703x over previous
"""Multi-head attention (12 heads, head_dim 64, RoPE, seq 1024) on 8 trn2 cores.

Sharding: pure data-parallel over the 16 (batch, row) units -> 2 per core.
No collectives. Each core runs the full per-unit attention.

v3: one global software pipeline across both (b,r) units. The ACT
engine's exp stream (~220us/core) is the co-bottleneck with the PE
(~250us/core), so emission interleaves them:

  lead-in: xT + v-proj + qk tile 0 of unit 0 (PE-dense, ~28us)
  then 24 attention iterations (2 units x 2 q-chunks x 6 head-pairs),
  each split into 4 segments emitting, in order:
    [fill unit]   next chunk from a work queue: remaining qk tiles of
                  unit 0, then the whole projection phase of unit 1,
                  plus pending out-projection chunks
    [pv chunk]    2 kt of PV+ones matmuls for the PREVIOUS iteration
    [scores grp]  4 scores matmuls (row groups alternate (0,0)/(64,0)
                  so adjacent pairs overlap) + 2 exp ACTs
  so the PE always has ready work while ACT drains the exp backlog.

  All matmul operands bf16 (FWL weight loads); psum fp32. QC=512.
  softmax: exp on ACT (scale=1/8, no max subtraction); sums broadcast
  across partitions by ones[128,64] matmuls into the same psum tile as
  PV; normalize = reciprocal_approx_fast + TT mult.

  biases: bq/bk applied in-kernel; bv/bo folded in on the host:
  out += bv @ Wo + bo (exact: sum(probs)=1).
  mask: all-ones fast path; any zero -> exact numpy fallback.
"""
from collections import deque

import numpy as np

H = 768
NH = 12
HD = 64
S = 1024
P = 128
DT = H // P          # 6 din/dout tiles
ST = S // P          # 8 seq tiles
BR = 2               # (b,r) units per core
NCORES = 8
QC = 512             # q-chunk
NQC = S // QC        # 2
ROPE_BASE = 10000.0

_CACHE = {}


def _rope_tables():
    inv = 1.0 / (ROPE_BASE ** (np.arange(0, HD, 2, dtype=np.float64) / HD))  # [32]
    t = np.arange(S, dtype=np.float64)
    f = np.outer(inv, t)                      # [32, S]
    cos2 = np.zeros((P, S), dtype=np.float32)
    sins = np.zeros((P, S), dtype=np.float32)
    c = np.cos(f).astype(np.float32)
    s = np.sin(f).astype(np.float32)
    for p in range(P):
        cos2[p] = c[p % 32]
        sins[p] = -s[p % 32] if (p % 64) < 32 else s[p % 32]
    return cos2, sins


def _legalize_waits(nc):
    """This walrus encodes at most one sync wait per instruction: hoist
    excess waits onto preceding same-engine NoOps."""
    import concourse.mybir as mybir

    n = 0
    for f in nc.m.functions:
        for blk in f.blocks:
            new = []
            for inst in blk.instructions:
                si = inst.sync_info
                waits = list(si.on_wait) if si and si.on_wait else []
                if len(waits) > 1:
                    for i, w in enumerate(waits[:-1]):
                        nop = mybir.InstNoOp(
                            name=f"{inst.name}-wn{i}", ins=[], outs=[],
                            sync_info=mybir.SyncInfo(on_wait=[w], on_update=[]))
                        nop.engine = inst.engine
                        new.append(nop)
                        n += 1
                    inst.sync_info = mybir.SyncInfo(
                        on_wait=[waits[-1]],
                        on_update=list(si.on_update) if si.on_update else [])
                new.append(inst)
            blk.instructions = new
    return n


def _build():
    import concourse.bass as bass
    import concourse.mybir as mybir
    import concourse.tile as tile

    F32 = mybir.dt.float32
    BF16 = mybir.dt.bfloat16
    Exp = mybir.ActivationFunctionType.Exp
    MUL = mybir.AluOpType.mult
    ADD = mybir.AluOpType.add

    nc = bass.Bass()
    xs = nc.dram_tensor("xs", [BR, S, H], BF16, kind="ExternalInput")
    wq = nc.dram_tensor("wq", [H, H], BF16, kind="ExternalInput")
    wk = nc.dram_tensor("wk", [H, H], BF16, kind="ExternalInput")
    wv = nc.dram_tensor("wv", [H, H], BF16, kind="ExternalInput")
    wo = nc.dram_tensor("wo", [H, H], BF16, kind="ExternalInput")
    bq = nc.dram_tensor("bq", [H], F32, kind="ExternalInput")
    bk = nc.dram_tensor("bk", [H], F32, kind="ExternalInput")
    cos2 = nc.dram_tensor("cos2", [P, S], BF16, kind="ExternalInput")
    sins = nc.dram_tensor("sins", [P, S], BF16, kind="ExternalInput")
    identm = nc.dram_tensor("identm", [P, P], BF16, kind="ExternalInput")
    onesb = nc.dram_tensor("onesb", [P, 64], BF16, kind="ExternalInput")
    out = nc.dram_tensor("out", [BR, S, H], F32, kind="ExternalOutput")

    with tile.TileContext(nc) as tc:
        with tc.tile_pool(name="const", bufs=1) as cpool, \
             tc.tile_pool(name="wpool", bufs=1) as wpool, \
             tc.tile_pool(name="xn", bufs=2) as xn_pool, \
             tc.tile_pool(name="xT", bufs=2) as xT_pool, \
             tc.tile_pool(name="qk", bufs=2) as qk_pool, \
             tc.tile_pool(name="rope", bufs=2) as rope_pool, \
             tc.tile_pool(name="vp", bufs=2) as v_pool, \
             tc.tile_pool(name="probs", bufs=2) as probs_pool, \
             tc.tile_pool(name="rec", bufs=2) as rec_pool, \
             tc.tile_pool(name="at", bufs=2) as at_pool, \
             tc.tile_pool(name="ot", bufs=2) as ot_pool, \
             tc.tile_pool(name="ps", bufs=3, space="PSUM") as ps_pool, \
             tc.tile_pool(name="pv", bufs=1, space="PSUM") as pv_pool:

            ident = cpool.tile([P, P], BF16, tag="ident")
            nc.sync.dma_start(ident[:], identm[:])
            ones64 = cpool.tile([P, 64], BF16, tag="ones")
            nc.sync.dma_start(ones64[:], onesb[:])
            cos_sb = cpool.tile([P, S], BF16, tag="cos")
            sin_sb = cpool.tile([P, S], BF16, tag="sin")
            nc.sync.dma_start(cos_sb[:], cos2[:])
            nc.sync.dma_start(sin_sb[:], sins[:])
            bq_sb = cpool.tile([P, DT], F32, tag="bq")
            bk_sb = cpool.tile([P, DT], F32, tag="bk")
            nc.sync.dma_start(bq_sb[:], bq.rearrange("(t p) -> p t", p=P))
            nc.sync.dma_start(bk_sb[:], bk.rearrange("(t p) -> p t", p=P))

            w_sb = {}
            for name, w in (("q", wq), ("k", wk), ("v", wv), ("o", wo)):
                w_sb[name] = wpool.tile([P, DT, H], BF16, tag=f"w{name}",
                                        name=f"w{name}")
                nc.sync.dma_start(
                    w_sb[name][:], w.rearrange("(t p) o -> p t o", p=P))

            # per-unit tile handles (pool tags rotate by allocation order)
            U = []
            for br in range(BR):
                xT = xT_pool.tile([P, DT, S], BF16, tag="xT", name=f"xT_{br}")
                v_sb = [v_pool.tile([P, H], BF16, tag=f"v{st}",
                                    name=f"v{st}_{br}") for st in range(ST)]
                qT = qk_pool.tile([P, DT, S], BF16, tag="qT", name=f"qT_{br}")
                kT = qk_pool.tile([P, DT, S], BF16, tag="kT", name=f"kT_{br}")
                U.append(dict(xT=xT, v=v_sb, q=qT, k=kT))

            bias_sb = {"q": bq_sb, "k": bk_sb}

            # ------------ emit helpers ------------
            def emit_xT_chunk(br, st):
                xT = U[br]["xT"]
                xn = xn_pool.tile([P, H], BF16, tag="xn", name="xn")
                nc.sync.dma_start(xn[:], xs[br, st * P:(st + 1) * P, :])
                pt = ps_pool.tile([P, 1024], BF16, tag="ps", name="pt")
                for dj in range(DT):
                    nc.tensor.transpose(pt[:, dj * P:(dj + 1) * P],
                                        xn[:, dj * P:(dj + 1) * P], ident)
                nc.vector.tensor_copy(
                    xT[:, :, st * P:(st + 1) * P],
                    pt[:, 0:DT * P].rearrange("p (t c) -> p t c", c=P))

            def emit_v_chunk(br, st):
                xT, vt = U[br]["xT"], U[br]["v"][st]
                pp = ps_pool.tile([P, 2, 512], F32, tag="ps", name="pp")
                for nb in range(2):
                    c0 = nb * 384
                    for dj in range(DT):
                        nc.tensor.matmul(
                            pp[:, nb, 0:384],
                            xT[:, dj, st * P:(st + 1) * P],
                            w_sb["v"][:, dj, c0:c0 + 384],
                            start=(dj == 0), stop=(dj == DT - 1))
                nc.vector.tensor_copy(
                    vt[:].rearrange("p (n c) -> p n c", n=2), pp[:, :, 0:384])

            def emit_qk_half(br, name, tt, half):
                xT, dst = U[br]["xT"], U[br][name]
                pp = ps_pool.tile([P, 512], F32, tag="ps", name="pp",
                                  uniquify=True)
                for dj in range(DT):
                    nc.tensor.matmul(
                        pp[:],
                        w_sb[name][:, dj, tt * P:(tt + 1) * P],
                        xT[:, dj, half * 512:(half + 1) * 512],
                        start=(dj == 0), stop=(dj == DT - 1))
                nc.vector.tensor_scalar_add(
                    dst[:, tt, half * 512:(half + 1) * 512], pp[:],
                    bias_sb[name][:, tt:tt + 1])
                if half == 1:
                    # RoPE: dst = dst*cos + swap(dst)*sins
                    sw = rope_pool.tile([P, S], BF16, tag="ropesw", name="sw")
                    for hh2 in range(2):
                        b0 = hh2 * 64
                        nc.sync.dma_start(sw[b0:b0 + 32, :],
                                          dst[b0 + 32:b0 + 64, tt, :])
                        nc.sync.dma_start(sw[b0 + 32:b0 + 64, :],
                                          dst[b0:b0 + 32, tt, :])
                    nc.vector.tensor_tensor(sw[:], sw[:], sin_sb[:], MUL)
                    nc.vector.tensor_tensor(dst[:, tt, :], dst[:, tt, :],
                                            cos_sb[:], MUL)
                    nc.vector.tensor_tensor(dst[:, tt, :], dst[:, tt, :],
                                            sw[:], ADD)

            def emit_scores_group(br, qc, hp, g, prs):
                q0 = qc * QC
                qT, kT = U[br]["q"], U[br]["k"]
                scs = [ps_pool.tile([P, 2, QC], F32, tag="ps",
                                    name=f"sc{i}") for i in range(2)]
                for i in range(2):
                    kt = 2 * g + i
                    for hh, base in ((0, 0), (1, 64)):
                        nc.tensor.matmul(
                            scs[hh][:, i, :],
                            kT[base:base + 64, hp, kt * P:(kt + 1) * P],
                            qT[base:base + 64, hp, q0:q0 + QC],
                            start=True, stop=True,
                            tile_position=(base, 0))
                for hh in range(2):
                    nc.scalar.activation(
                        prs[hh][:, 2 * g:2 * g + 2, :], scs[hh][:],
                        Exp, scale=0.125)

            def emit_pv_chunk(br, hp, prs, pvt, g):
                for i in range(2):
                    kt = 2 * g + i
                    for hh in range(2):
                        nc.tensor.matmul(
                            pvt[hh * 64:(hh + 1) * 64, 0, :],
                            U[br]["v"][kt][:, (2 * hp + hh) * HD:
                                           (2 * hp + hh + 1) * HD],
                            prs[hh][:, kt, :],
                            start=(kt == 0), stop=(kt == ST - 1),
                            tile_position=(0, hh * 64))
                    for hh in range(2):
                        nc.tensor.matmul(
                            pvt[hh * 64:(hh + 1) * 64, 1, :],
                            ones64[:], prs[hh][:, kt, :],
                            start=(kt == 0), stop=(kt == ST - 1),
                            tile_position=(0, hh * 64))

            def emit_normalize(hp, pvt, at):
                rec = rec_pool.tile([P, QC], F32, tag="rec", name="rec")
                nc.vector.reciprocal(rec[:], pvt[:, 1, :])
                att = at_pool.tile([P, QC], BF16, tag=f"at{hp}", name=f"at{hp}")
                at[hp] = att
                nc.vector.tensor_tensor(att[:], pvt[:, 0, :], rec[:], MUL)

            def emit_outproj_chunk(br, qc, at, sc2):
                q0 = qc * QC
                po = ps_pool.tile([P, 2, 512], F32, tag="ps", name="po")
                for nb in range(2):
                    c0 = nb * 384
                    for dj in range(DT):
                        nc.tensor.matmul(
                            po[:, nb, 0:384],
                            at[dj][:, sc2 * P:(sc2 + 1) * P],
                            w_sb["o"][:, dj, c0:c0 + 384],
                            start=(dj == 0), stop=(dj == DT - 1))
                ot = ot_pool.tile([P, H], F32, tag="ot", name="ot")
                nc.vector.tensor_copy(
                    ot[:].rearrange("p (n c) -> p n c", n=2), po[:, :, 0:384])
                r0 = q0 + sc2 * P
                nc.sync.dma_start(out[br, r0:r0 + P, :], ot[:])

            # ------------ lead-in: unit 0 xT + v + qk tile 0 ------------
            for st in range(ST):
                emit_xT_chunk(0, st)
            for st in range(ST):
                emit_v_chunk(0, st)
            for name in ("q", "k"):
                for half in range(2):
                    emit_qk_half(0, name, 0, half)

            # ------------ fill-unit work queue ------------
            fills = deque()
            for tt in range(1, DT):
                for name in ("q", "k"):
                    for half in range(2):
                        fills.append((emit_qk_half, (0, name, tt, half)))
            for st in range(ST):
                fills.append((emit_xT_chunk, (1, st)))
            for st in range(ST):
                fills.append((emit_v_chunk, (1, st)))
            for tt in range(DT):
                for name in ("q", "k"):
                    for half in range(2):
                        fills.append((emit_qk_half, (1, name, tt, half)))

            # ------------ global pipelined attention ------------
            iters = [(br, qc, hp) for br in range(BR)
                     for qc in range(NQC) for hp in range(DT)]
            at_cur = [{} for _ in range(BR)]
            prev = None
            pending_op = deque()   # out-projection chunks, 1 per segment
            for (br, qc, hp) in iters:
                prs = [probs_pool.tile([P, ST, QC], BF16, tag=f"pr{hh}",
                                       name=f"pr{hh}") for hh in range(2)]
                if prev is not None:
                    pvt = pv_pool.tile([P, 2, QC], F32, tag="pv", name="pvt")
                for g in range(4):
                    if pending_op:
                        f, a = pending_op.popleft()
                        f(*a)
                    elif fills:
                        f, a = fills.popleft()
                        f(*a)
                    if prev is not None:
                        emit_pv_chunk(prev[0], prev[2], prev[3], pvt, g)
                    emit_scores_group(br, qc, hp, g, prs)
                if prev is not None:
                    pbr, pqc, php, pprs = prev
                    emit_normalize(php, pvt, at_cur[pbr])
                    if php == DT - 1:
                        for sc2 in range(QC // P):
                            pending_op.append(
                                (emit_outproj_chunk,
                                 (pbr, pqc, dict(at_cur[pbr]), sc2)))
                prev = (br, qc, hp, prs)
            # drain: final pv + normalize + out-projection
            pbr, pqc, php, pprs = prev
            pvt = pv_pool.tile([P, 2, QC], F32, tag="pv", name="pvt")
            for g in range(4):
                if pending_op:
                    f, a = pending_op.popleft()
                    f(*a)
                emit_pv_chunk(pbr, php, pprs, pvt, g)
            emit_normalize(php, pvt, at_cur[pbr])
            while pending_op:
                f, a = pending_op.popleft()
                f(*a)
            for sc2 in range(QC // P):
                emit_outproj_chunk(pbr, pqc, at_cur[pbr], sc2)

    _legalize_waits(nc)
    return nc


def _get_nc():
    if "nc" not in _CACHE:
        _CACHE["nc"] = _build()
    return _CACHE["nc"]


def _numpy_reference(x, Wq, bq, Wk, bk, Wv, bv, Wo, bo, mask):
    b, r, s, d = x.shape
    inv = 1.0 / (ROPE_BASE ** (np.arange(0, HD, 2, dtype=np.float32) / HD))
    t = np.arange(s, dtype=np.float32)
    f = np.outer(t, inv)
    emb = np.concatenate([f, f], axis=-1)
    cos, sin = np.cos(emb), np.sin(emb)

    def proj(W, bvec):
        y = x @ W + bvec
        return y.reshape(b, r, s, NH, HD).transpose(0, 1, 3, 2, 4)

    def rot(z):
        z1, z2 = z[..., :HD // 2], z[..., HD // 2:]
        return np.concatenate([-z2, z1], axis=-1)

    q = proj(Wq, bq)
    k = proj(Wk, bk)
    v = proj(Wv, bv)
    q = q * cos + rot(q) * sin
    k = k * cos + rot(k) * sin
    scores = np.einsum("brhqd,brhkd->brhqk", q, k) / np.sqrt(np.float32(HD))
    scores = np.where(mask == 0, -np.inf, scores)
    m = scores.max(axis=-1, keepdims=True)
    e = np.exp(scores - m)
    probs = e / e.sum(axis=-1, keepdims=True)
    o = np.einsum("brhqk,brhkd->brhqd", probs, v)
    o = o.transpose(0, 1, 3, 2, 4).reshape(b, r, s, d)
    return (o @ Wo + bo).astype(np.float32)


def _run(inputs, trace=False):
    import ml_dtypes
    from concourse.bass_utils import run_bass_kernel_spmd

    BF = ml_dtypes.bfloat16
    x = np.asarray(inputs["x"], dtype=np.float32)
    Wq = np.ascontiguousarray(np.asarray(inputs["Wq"], dtype=np.float32))
    Wk = np.ascontiguousarray(np.asarray(inputs["Wk"], dtype=np.float32))
    Wv = np.ascontiguousarray(np.asarray(inputs["Wv"], dtype=np.float32))
    Wo = np.ascontiguousarray(np.asarray(inputs["Wo"], dtype=np.float32))
    bq = np.asarray(inputs["bq"], dtype=np.float32)
    bk = np.asarray(inputs["bk"], dtype=np.float32)
    bv = np.asarray(inputs["bv"], dtype=np.float32)
    bo = np.asarray(inputs["bo"], dtype=np.float32)

    xf = np.ascontiguousarray(x.reshape(NCORES * BR, S, H).astype(BF))
    cos2, sins = _rope_tables()
    cos2 = cos2.astype(BF)
    sins = sins.astype(BF)
    identm = np.eye(P, dtype=np.float32).astype(BF)
    onesb = np.ones((P, 64), dtype=BF)
    wqb, wkb, wvb, wob = (np.ascontiguousarray(w.astype(BF))
                          for w in (Wq, Wk, Wv, Wo))
    nc = _get_nc()
    in_maps = []
    for c in range(NCORES):
        in_maps.append(dict(
            xs=np.ascontiguousarray(xf[c * BR:(c + 1) * BR]),
            wq=wqb, wk=wkb, wv=wvb, wo=wob, bq=bq, bk=bk,
            cos2=cos2, sins=sins, identm=identm, onesb=onesb))
    res = run_bass_kernel_spmd(nc, in_maps, core_ids=list(range(NCORES)),
                               trace=trace)
    outs = np.concatenate([r["out"] for r in res.results], axis=0)
    out = outs.reshape(2, NCORES * BR // 2, S, H)
    out = out + (bv @ Wo + bo)
    return out.astype(np.float32), res


def kernel(**inputs):
    mask = np.asarray(inputs["mask"])
    if not np.all(mask != 0):
        return _numpy_reference(
            x=np.asarray(inputs["x"], np.float32),
            Wq=np.asarray(inputs["Wq"], np.float32),
            bq=np.asarray(inputs["bq"], np.float32),
            Wk=np.asarray(inputs["Wk"], np.float32),
            bk=np.asarray(inputs["bk"], np.float32),
            Wv=np.asarray(inputs["Wv"], np.float32),
            bv=np.asarray(inputs["bv"], np.float32),
            Wo=np.asarray(inputs["Wo"], np.float32),
            bo=np.asarray(inputs["bo"], np.float32),
            mask=mask)
    out, _ = _run(inputs, trace=False)
    return out


# revision 12
# speedup vs baseline: 1.5121x; 1.0021x over previous
"""Multi-head attention (12 heads, head_dim 64, RoPE, seq 1024) on 8 trn2 cores.

Sharding: pure data-parallel over the 16 (batch, row) units -> 2 per core.
No collectives. Each core runs the full per-unit attention.

v3: one global software pipeline across both (b,r) units. The ACT
engine's exp stream (~220us/core) is the co-bottleneck with the PE
(~250us/core), so emission interleaves them:

  lead-in: xT + v-proj + qk tile 0 of unit 0 (PE-dense, ~28us)
  then 24 attention iterations (2 units x 2 q-chunks x 6 head-pairs),
  each split into 4 segments emitting, in order:
    [fill unit]   next chunk from a work queue: remaining qk tiles of
                  unit 0, then the whole projection phase of unit 1,
                  plus pending out-projection chunks
    [pv chunk]    2 kt of PV+ones matmuls for the PREVIOUS iteration
    [scores grp]  4 scores matmuls (row groups alternate (0,0)/(64,0)
                  so adjacent pairs overlap) + 2 exp ACTs
  so the PE always has ready work while ACT drains the exp backlog.

  All matmul operands bf16 (FWL weight loads); psum fp32. QC=512.
  softmax: exp on ACT (scale=1/8, no max subtraction); sums broadcast
  across partitions by ones[128,64] matmuls into the same psum tile as
  PV; normalize = reciprocal_approx_fast + TT mult.

  biases: bq/bk applied in-kernel; bv/bo folded in on the host:
  out += bv @ Wo + bo (exact: sum(probs)=1).
  mask: all-ones fast path; any zero -> exact numpy fallback.
"""
from collections import deque

import numpy as np

H = 768
NH = 12
HD = 64
S = 1024
P = 128
DT = H // P          # 6 din/dout tiles
ST = S // P          # 8 seq tiles
BR = 2               # (b,r) units per core
NCORES = 8
QC = 512             # q-chunk
NQC = S // QC        # 2
ROPE_BASE = 10000.0

_CACHE = {}


def _rope_tables():
    inv = 1.0 / (ROPE_BASE ** (np.arange(0, HD, 2, dtype=np.float64) / HD))  # [32]
    t = np.arange(S, dtype=np.float64)
    f = np.outer(inv, t)                      # [32, S]
    cos2 = np.zeros((P, S), dtype=np.float32)
    sins = np.zeros((P, S), dtype=np.float32)
    c = np.cos(f).astype(np.float32)
    s = np.sin(f).astype(np.float32)
    for p in range(P):
        cos2[p] = c[p % 32]
        sins[p] = -s[p % 32] if (p % 64) < 32 else s[p % 32]
    return cos2, sins


def _legalize_waits(nc):
    """This walrus encodes at most one sync wait per instruction: hoist
    excess waits onto preceding same-engine NoOps."""
    import concourse.mybir as mybir

    n = 0
    for f in nc.m.functions:
        for blk in f.blocks:
            new = []
            for inst in blk.instructions:
                si = inst.sync_info
                waits = list(si.on_wait) if si and si.on_wait else []
                if len(waits) > 1:
                    for i, w in enumerate(waits[:-1]):
                        nop = mybir.InstNoOp(
                            name=f"{inst.name}-wn{i}", ins=[], outs=[],
                            sync_info=mybir.SyncInfo(on_wait=[w], on_update=[]))
                        nop.engine = inst.engine
                        new.append(nop)
                        n += 1
                    inst.sync_info = mybir.SyncInfo(
                        on_wait=[waits[-1]],
                        on_update=list(si.on_update) if si.on_update else [])
                new.append(inst)
            blk.instructions = new
    return n


def _build():
    import concourse.bass as bass
    import concourse.mybir as mybir
    import concourse.tile as tile

    F32 = mybir.dt.float32
    BF16 = mybir.dt.bfloat16
    Exp = mybir.ActivationFunctionType.Exp
    MUL = mybir.AluOpType.mult
    ADD = mybir.AluOpType.add

    nc = bass.Bass()
    xs = nc.dram_tensor("xs", [BR, S, H], BF16, kind="ExternalInput")
    wq = nc.dram_tensor("wq", [H, H], BF16, kind="ExternalInput")
    wk = nc.dram_tensor("wk", [H, H], BF16, kind="ExternalInput")
    wv = nc.dram_tensor("wv", [H, H], BF16, kind="ExternalInput")
    wo = nc.dram_tensor("wo", [H, H], BF16, kind="ExternalInput")
    bq = nc.dram_tensor("bq", [H], F32, kind="ExternalInput")
    bk = nc.dram_tensor("bk", [H], F32, kind="ExternalInput")
    cos2 = nc.dram_tensor("cos2", [P, S], BF16, kind="ExternalInput")
    sins = nc.dram_tensor("sins", [P, S], BF16, kind="ExternalInput")
    identm = nc.dram_tensor("identm", [P, P], BF16, kind="ExternalInput")
    onesb = nc.dram_tensor("onesb", [P, 64], BF16, kind="ExternalInput")
    out = nc.dram_tensor("out", [BR, S, H], F32, kind="ExternalOutput")

    with tile.TileContext(nc) as tc:
        with tc.tile_pool(name="const", bufs=1) as cpool, \
             tc.tile_pool(name="wpool", bufs=1) as wpool, \
             tc.tile_pool(name="xn", bufs=1) as xn_pool, \
             tc.tile_pool(name="xT", bufs=2) as xT_pool, \
             tc.tile_pool(name="qk", bufs=2) as qk_pool, \
             tc.tile_pool(name="rope", bufs=1) as rope_pool, \
             tc.tile_pool(name="vp", bufs=2) as v_pool, \
             tc.tile_pool(name="probs", bufs=2) as probs_pool, \
             tc.tile_pool(name="rec", bufs=1) as rec_pool, \
             tc.tile_pool(name="at", bufs=2) as at_pool, \
             tc.tile_pool(name="ot", bufs=2) as ot_pool, \
             tc.tile_pool(name="ps", bufs=2, space="PSUM") as ps_pool, \
             tc.tile_pool(name="pv", bufs=2, space="PSUM") as pv_pool:

            ident = cpool.tile([P, P], BF16, tag="ident")
            nc.sync.dma_start(ident[:], identm[:])
            ones64 = cpool.tile([P, 64], BF16, tag="ones")
            nc.sync.dma_start(ones64[:], onesb[:])
            cos_sb = cpool.tile([P, S], BF16, tag="cos")
            sin_sb = cpool.tile([P, S], BF16, tag="sin")
            nc.sync.dma_start(cos_sb[:], cos2[:])
            nc.sync.dma_start(sin_sb[:], sins[:])
            bq_sb = cpool.tile([P, DT], F32, tag="bq")
            bk_sb = cpool.tile([P, DT], F32, tag="bk")
            nc.sync.dma_start(bq_sb[:], bq.rearrange("(t p) -> p t", p=P))
            nc.sync.dma_start(bk_sb[:], bk.rearrange("(t p) -> p t", p=P))

            w_sb = {}
            for name, w in (("q", wq), ("k", wk), ("v", wv), ("o", wo)):
                w_sb[name] = wpool.tile([P, DT, H], BF16, tag=f"w{name}",
                                        name=f"w{name}")
                nc.sync.dma_start(
                    w_sb[name][:], w.rearrange("(t p) o -> p t o", p=P))

            # per-unit tile handles (pool tags rotate by allocation order)
            U = []
            for br in range(BR):
                xT = xT_pool.tile([P, DT, S], BF16, tag="xT", name=f"xT_{br}")
                v_sb = [v_pool.tile([P, H], BF16, tag=f"v{st}",
                                    name=f"v{st}_{br}") for st in range(ST)]
                qT = qk_pool.tile([P, DT, S], BF16, tag="qT", name=f"qT_{br}")
                kT = qk_pool.tile([P, DT, S], BF16, tag="kT", name=f"kT_{br}")
                U.append(dict(xT=xT, v=v_sb, q=qT, k=kT))

            bias_sb = {"q": bq_sb, "k": bk_sb}

            # warm the exp table set during the lead-in DMAs
            wup = cpool.tile([1, 4], F32, tag="wup")
            nc.scalar.activation(wup[:], bq_sb[0:1, 0:4],
                                 mybir.ActivationFunctionType.Exp)

            # ------------ emit helpers ------------
            xn_tiles = {}

            def emit_xn_load(br, st):
                t = xn_pool.tile([P, H], BF16, tag=f"xn{st}", name=f"xn{st}")
                nc.sync.dma_start(t[:], xs[br, st * P:(st + 1) * P, :])
                xn_tiles[(br, st)] = t

            def emit_xT_chunk(br, st):
                xT = U[br]["xT"]
                xn = xn_tiles.pop((br, st))
                pt = ps_pool.tile([P, 1024], BF16, tag="ps", name="pt")
                for dj in range(DT):
                    nc.tensor.transpose(pt[:, dj * P:(dj + 1) * P],
                                        xn[:, dj * P:(dj + 1) * P], ident)
                nc.vector.tensor_copy(
                    xT[:, :, st * P:(st + 1) * P],
                    pt[:, 0:DT * P].rearrange("p (t c) -> p t c", c=P))

            def emit_v_chunk(br, st):
                xT, vt = U[br]["xT"], U[br]["v"][st]
                pp = ps_pool.tile([P, 2, 512], F32, tag="ps", name="pp")
                for nb in range(2):
                    c0 = nb * 384
                    for dj in range(DT):
                        nc.tensor.matmul(
                            pp[:, nb, 0:384],
                            xT[:, dj, st * P:(st + 1) * P],
                            w_sb["v"][:, dj, c0:c0 + 384],
                            start=(dj == 0), stop=(dj == DT - 1))
                nc.vector.tensor_copy(
                    vt[:].rearrange("p (n c) -> p n c", n=2), pp[:, :, 0:384])

            def emit_qk_half(br, name, tt, half):
                xT, dst = U[br]["xT"], U[br][name]
                pp = ps_pool.tile([P, 512], F32, tag="ps", name="pp",
                                  uniquify=True)
                for dj in range(DT):
                    nc.tensor.matmul(
                        pp[:],
                        w_sb[name][:, dj, tt * P:(tt + 1) * P],
                        xT[:, dj, half * 512:(half + 1) * 512],
                        start=(dj == 0), stop=(dj == DT - 1))
                nc.vector.tensor_scalar_add(
                    dst[:, tt, half * 512:(half + 1) * 512], pp[:],
                    bias_sb[name][:, tt:tt + 1])
                if half == 1:
                    # RoPE: dst = dst*cos + swap(dst)*sins
                    sw = rope_pool.tile([P, S], BF16, tag="ropesw", name="sw")
                    for hh2 in range(2):
                        b0 = hh2 * 64
                        nc.sync.dma_start(sw[b0:b0 + 32, :],
                                          dst[b0 + 32:b0 + 64, tt, :])
                        nc.sync.dma_start(sw[b0 + 32:b0 + 64, :],
                                          dst[b0:b0 + 32, tt, :])
                    nc.vector.tensor_tensor(sw[:], sw[:], sin_sb[:], MUL)
                    nc.vector.tensor_tensor(dst[:, tt, :], dst[:, tt, :],
                                            cos_sb[:], MUL)
                    nc.vector.tensor_tensor(dst[:, tt, :], dst[:, tt, :],
                                            sw[:], ADD)

            def emit_scores_kt(br, qc, hp, kt, prs):
                q0 = qc * QC
                qT, kT = U[br]["q"], U[br]["k"]
                sc = ps_pool.tile([P, 2, QC], F32, tag="ps", name="sc")
                for hh, base in ((0, 0), (1, 64)):
                    nc.tensor.matmul(
                        sc[:, hh, :],
                        kT[base:base + 64, hp, kt * P:(kt + 1) * P],
                        qT[base:base + 64, hp, q0:q0 + QC],
                        start=True, stop=True,
                        tile_position=(base, 0))
                nc.scalar.activation(prs[:, kt, :, :], sc[:], Exp, scale=0.125)

            def emit_pv_kt(br, hp, prs, pvt, kt):
                for hh in range(2):
                    nc.tensor.matmul(
                        pvt[hh * 64:(hh + 1) * 64, 0, :],
                        U[br]["v"][kt][:, (2 * hp + hh) * HD:
                                       (2 * hp + hh + 1) * HD],
                        prs[:, kt, hh, :],
                        start=(kt == 0), stop=(kt == ST - 1),
                        tile_position=(0, hh * 64))
                for hh in range(2):
                    nc.tensor.matmul(
                        pvt[hh * 64:(hh + 1) * 64, 1, :],
                        ones64[:], prs[:, kt, hh, :],
                        start=(kt == 0), stop=(kt == ST - 1),
                        tile_position=(0, hh * 64))

            def emit_normalize(hp, pvt, at):
                rec = rec_pool.tile([P, QC], F32, tag="rec", name="rec")
                nc.vector.reciprocal(rec[:], pvt[:, 1, :])
                att = at_pool.tile([P, QC], BF16, tag=f"at{hp}", name=f"at{hp}")
                at[hp] = att
                nc.vector.tensor_tensor(att[:], pvt[:, 0, :], rec[:], MUL)

            def emit_outproj_chunk(br, qc, at, sc2):
                q0 = qc * QC
                po = ps_pool.tile([P, 2, 512], F32, tag="ps", name="po")
                for nb in range(2):
                    c0 = nb * 384
                    for dj in range(DT):
                        nc.tensor.matmul(
                            po[:, nb, 0:384],
                            at[dj][:, sc2 * P:(sc2 + 1) * P],
                            w_sb["o"][:, dj, c0:c0 + 384],
                            start=(dj == 0), stop=(dj == DT - 1))
                ot = ot_pool.tile([P, H], F32, tag="ot", name="ot")
                nc.vector.tensor_copy(
                    ot[:].rearrange("p (n c) -> p n c", n=2), po[:, :, 0:384])
                r0 = q0 + sc2 * P
                nc.sync.dma_start(out[br, r0:r0 + P, :], ot[:])

            # ------------ lead-in: unit 0 xT + v + qk tile 0 ------------
            for st in range(ST):
                emit_xn_load(0, st)
            for st in range(ST):
                emit_xT_chunk(0, st)
            for st in range(ST):
                emit_v_chunk(0, st)
            for name in ("q", "k"):
                for half in range(2):
                    emit_qk_half(0, name, 0, half)
            for st in range(ST):
                emit_xn_load(1, st)

            # ------------ fill-unit work queue ------------
            fills = deque()
            for tt in range(1, DT):
                for name in ("q", "k"):
                    for half in range(2):
                        fills.append((emit_qk_half, (0, name, tt, half)))
            for st in range(ST):
                fills.append((emit_xT_chunk, (1, st)))
            for st in range(ST):
                fills.append((emit_v_chunk, (1, st)))
            for tt in range(DT):
                for name in ("q", "k"):
                    for half in range(2):
                        fills.append((emit_qk_half, (1, name, tt, half)))

            # ------------ global pipelined attention ------------
            iters = [(br, qc, hp) for br in range(BR)
                     for qc in range(NQC) for hp in range(DT)]
            at_cur = [{} for _ in range(BR)]
            prev = None
            pending_op = deque()   # out-projection chunks
            for (br, qc, hp) in iters:
                prs = probs_pool.tile([P, ST, 2, QC], BF16, tag="pr",
                                      name="pr")
                if prev is not None:
                    pvt = pv_pool.tile([P, 2, QC], F32, tag="pv", name="pvt")
                for kt in range(ST):
                    if kt % 2 == 0:
                        if pending_op:
                            f, a = pending_op.popleft()
                            f(*a)
                        elif fills:
                            f, a = fills.popleft()
                            f(*a)
                    if prev is not None:
                        emit_pv_kt(prev[0], prev[2], prev[3], pvt, kt)
                    emit_scores_kt(br, qc, hp, kt, prs)
                if prev is not None:
                    pbr, pqc, php, pprs = prev
                    emit_normalize(php, pvt, at_cur[pbr])
                    if php == DT - 1:
                        for sc2 in range(QC // P):
                            pending_op.append(
                                (emit_outproj_chunk,
                                 (pbr, pqc, dict(at_cur[pbr]), sc2)))
                prev = (br, qc, hp, prs)
            # drain: final pv + normalize + out-projection
            pbr, pqc, php, pprs = prev
            pvt = pv_pool.tile([P, 2, QC], F32, tag="pv", name="pvt")
            for kt in range(ST):
                if pending_op:
                    f, a = pending_op.popleft()
                    f(*a)
                emit_pv_kt(pbr, php, pprs, pvt, kt)
            emit_normalize(php, pvt, at_cur[pbr])
            while pending_op:
                f, a = pending_op.popleft()
                f(*a)
            for sc2 in range(QC // P):
                emit_outproj_chunk(pbr, pqc, at_cur[pbr], sc2)

    _legalize_waits(nc)
    return nc


def _get_nc():
    if "nc" not in _CACHE:
        _CACHE["nc"] = _build()
    return _CACHE["nc"]


def _numpy_reference(x, Wq, bq, Wk, bk, Wv, bv, Wo, bo, mask):
    b, r, s, d = x.shape
    inv = 1.0 / (ROPE_BASE ** (np.arange(0, HD, 2, dtype=np.float32) / HD))
    t = np.arange(s, dtype=np.float32)
    f = np.outer(t, inv)
    emb = np.concatenate([f, f], axis=-1)
    cos, sin = np.cos(emb), np.sin(emb)

    def proj(W, bvec):
        y = x @ W + bvec
        return y.reshape(b, r, s, NH, HD).transpose(0, 1, 3, 2, 4)

    def rot(z):
        z1, z2 = z[..., :HD // 2], z[..., HD // 2:]
        return np.concatenate([-z2, z1], axis=-1)

    q = proj(Wq, bq)
    k = proj(Wk, bk)
    v = proj(Wv, bv)
    q = q * cos + rot(q) * sin
    k = k * cos + rot(k) * sin
    scores = np.einsum("brhqd,brhkd->brhqk", q, k) / np.sqrt(np.float32(HD))
    scores = np.where(mask == 0, -np.inf, scores)
    m = scores.max(axis=-1, keepdims=True)
    e = np.exp(scores - m)
    probs = e / e.sum(axis=-1, keepdims=True)
    o = np.einsum("brhqk,brhkd->brhqd", probs, v)
    o = o.transpose(0, 1, 3, 2, 4).reshape(b, r, s, d)
    return (o @ Wo + bo).astype(np.float32)


def _run(inputs, trace=False):
    import ml_dtypes
    from concourse.bass_utils import run_bass_kernel_spmd

    BF = ml_dtypes.bfloat16
    x = np.asarray(inputs["x"], dtype=np.float32)
    Wq = np.ascontiguousarray(np.asarray(inputs["Wq"], dtype=np.float32))
    Wk = np.ascontiguousarray(np.asarray(inputs["Wk"], dtype=np.float32))
    Wv = np.ascontiguousarray(np.asarray(inputs["Wv"], dtype=np.float32))
    Wo = np.ascontiguousarray(np.asarray(inputs["Wo"], dtype=np.float32))
    bq = np.asarray(inputs["bq"], dtype=np.float32)
    bk = np.asarray(inputs["bk"], dtype=np.float32)
    bv = np.asarray(inputs["bv"], dtype=np.float32)
    bo = np.asarray(inputs["bo"], dtype=np.float32)

    xf = np.ascontiguousarray(x.reshape(NCORES * BR, S, H).astype(BF))
    cos2, sins = _rope_tables()
    cos2 = cos2.astype(BF)
    sins = sins.astype(BF)
    identm = np.eye(P, dtype=np.float32).astype(BF)
    onesb = np.ones((P, 64), dtype=BF)
    wqb, wkb, wvb, wob = (np.ascontiguousarray(w.astype(BF))
                          for w in (Wq, Wk, Wv, Wo))
    nc = _get_nc()
    in_maps = []
    for c in range(NCORES):
        in_maps.append(dict(
            xs=np.ascontiguousarray(xf[c * BR:(c + 1) * BR]),
            wq=wqb, wk=wkb, wv=wvb, wo=wob, bq=bq, bk=bk,
            cos2=cos2, sins=sins, identm=identm, onesb=onesb))
    res = run_bass_kernel_spmd(nc, in_maps, core_ids=list(range(NCORES)),
                               trace=trace)
    outs = np.concatenate([r["out"] for r in res.results], axis=0)
    out = outs.reshape(2, NCORES * BR // 2, S, H)
    out = out + (bv @ Wo + bo)
    return out.astype(np.float32), res


def kernel(**inputs):
    mask = np.asarray(inputs["mask"])
    if not np.all(mask != 0):
        return _numpy_reference(
            x=np.asarray(inputs["x"], np.float32),
            Wq=np.asarray(inputs["Wq"], np.float32),
            bq=np.asarray(inputs["bq"], np.float32),
            Wk=np.asarray(inputs["Wk"], np.float32),
            bk=np.asarray(inputs["bk"], np.float32),
            Wv=np.asarray(inputs["Wv"], np.float32),
            bv=np.asarray(inputs["bv"], np.float32),
            Wo=np.asarray(inputs["Wo"], np.float32),
            bo=np.asarray(inputs["bo"], np.float32),
            mask=mask)
    out, _ = _run(inputs, trace=False)
    return out
